# revision 1
# baseline (speedup 1.0000x reference)
"""CronRootAttention (causal sqrt-N sparse attention + GQA projections) on 8 TRN2 cores.

Sharding: pure sequence shard — each core owns 256 queries, computes all 16
heads for them. Weights are replicated; kv projections computed per-core for
the local 384-key span plus the 44 shared strided keys.

v2 dataflow changes vs baseline:
  - Causal masks are ADDITIVE (-400 on invalid) and preloaded into the score
    PSUM via PE identity matmuls (start=True); score matmuls accumulate on
    top (start=False) so exp reads masked scores directly — no DVE mask
    multiplies at all.
  - Softmax denominators (ones-column rows of the PV psum) are normalized via
    DVE reciprocal + gpsimd partition_broadcast + tensor_mul; the PE broadcast
    matmul and its PSUM->SBUF copy are gone.
  - PSUM->SBUF projection copies rotate across Activation/DVE/Pool engines.
  - Weight/mask DMAs are batched (3D APs) to cut SP issue serialization.
"""

import math
import sys

sys.path.insert(0, "/opt/trn_rl_repo")

import numpy as np
import concourse.bass as bass
import concourse.tile as tile
from concourse import bacc, mybir
from concourse.bass_utils import run_bass_kernel_spmd

F32 = mybir.dt.float32
BF16 = mybir.dt.bfloat16
EXP = mybir.ActivationFunctionType.Exp
COPY = mybir.ActivationFunctionType.Copy
import os
import ml_dtypes

MM_DT = BF16
NP_DT = ml_dtypes.bfloat16

PSS_BUFS = int(os.environ.get("PSS_BUFS", "3"))
PSSW_BUFS = int(os.environ.get("PSSW_BUFS", "1"))
PSPV_BUFS = int(os.environ.get("PSPV_BUFS", "3"))

# Problem constants (hardcoded per contract).
B, S, D = 1, 2048, 1024
H, H_KV, HD = 16, 4, 64
W = int(math.ceil(math.sqrt(S)))  # 46
NCORES = 8
SQ = S // NCORES  # 256 queries per core
SKV = 384  # local key span: [qs-128, qs+256)
SIDX = np.arange(W - 1, S, W)  # strided key positions
NS = len(SIDX)  # 44
KT = D // 128  # 8 contraction k-tiles
MASKV = -400.0  # additive mask value; exp(0.125*(qk-400)) == 0 in f32/bf16
# jt1 scores psum: heads packed at these free offsets so no matmul output
# crosses a 512-f32 PSUM bank boundary ([0:192],[192:384] | [512:704],[704:896]).
J1OFF = (0, 192, 512, 704)
# i-window (in local query coords) with valid local-attention pairs per j-tile.
WIN = ((0, 64), (0, 192), (128, 256))


def build_nc():
    nc = bacc.Bacc("TRN2", target_bir_lowering=False, debug=False, num_devices=1)
    xkv = nc.dram_tensor("xkv", [D, SKV], MM_DT, kind="ExternalInput").ap()
    xs = nc.dram_tensor("xs", [D, NS], MM_DT, kind="ExternalInput").ap()
    wq = nc.dram_tensor("wq", [D, D], MM_DT, kind="ExternalInput").ap()
    wk = nc.dram_tensor("wk", [D, 256], MM_DT, kind="ExternalInput").ap()
    wv = nc.dram_tensor("wv", [D, 260], MM_DT, kind="ExternalInput").ap()
    wo = nc.dram_tensor("wo", [D, D], MM_DT, kind="ExternalInput").ap()
    m0 = nc.dram_tensor("m0", [128, 256], MM_DT, kind="ExternalInput").ap()
    m1 = nc.dram_tensor("m1", [128, 768], MM_DT, kind="ExternalInput").ap()
    m2 = nc.dram_tensor("m2", [128, 512], MM_DT, kind="ExternalInput").ap()
    ms = nc.dram_tensor("ms", [NS, 1024], MM_DT, kind="ExternalInput").ap()
    ident = nc.dram_tensor("ident", [128, 128], MM_DT, kind="ExternalInput").ap()
    y = nc.dram_tensor("y", [SQ, D], MM_DT, kind="ExternalOutput").ap()

    with tile.TileContext(nc) as tc:
        with (
            tc.tile_pool(name="consts", bufs=1) as consts,
            tc.tile_pool(name="work", bufs=1) as work,
        ):
            # ---- resident SBUF tensors ----
            xkv_sb = consts.tile([128, KT, SKV], MM_DT)
            xs_sb = consts.tile([128, KT, NS], MM_DT)
            wk_sb = consts.tile([128, KT, 256], MM_DT)
            wv_sb = consts.tile([128, KT, 260], MM_DT)
            wq_sb = consts.tile([128, KT, D], MM_DT)
            wo_sb = consts.tile([128, KT, D], MM_DT)
            m0_sb = consts.tile([128, 256], MM_DT)
            m1_sb = consts.tile([128, 768], MM_DT)
            m2_sb = consts.tile([128, 512], MM_DT)
            ms_sb = consts.tile([NS, 1024], MM_DT)
            id_sb = consts.tile([128, 128], MM_DT)
            # batched loads ordered/split so the first matmuls start ASAP:
            # kt0-1 slices of xkv/wk first, then the rest, then later-phase data.
            xkv_r = xkv.rearrange("(kt p) s -> p kt s", p=128)
            wk_r = wk.rearrange("(kt p) o -> p kt o", p=128)
            wq_r = wq.rearrange("(h kt p) o -> p h kt o", p=128, h=2)
            nc.sync.dma_start(out=xkv_sb[:, 0:2, :], in_=xkv_r[:, 0:2])
            nc.sync.dma_start(out=wk_sb[:, 0:2, :], in_=wk_r[:, 0:2])
            nc.sync.dma_start(out=xkv_sb[:, 2:5, :], in_=xkv_r[:, 2:5])
            nc.sync.dma_start(out=wk_sb[:, 2:8, :], in_=wk_r[:, 2:8])
            nc.sync.dma_start(out=xkv_sb[:, 5:8, :], in_=xkv_r[:, 5:8])
            nc.sync.dma_start(
                out=wv_sb[:], in_=wv.rearrange("(kt p) o -> p kt o", p=128)
            )
            nc.sync.dma_start(out=wq_sb[:, 0:4, :], in_=wq_r[:, 0])
            nc.sync.dma_start(out=wq_sb[:, 4:8, :], in_=wq_r[:, 1])
            nc.sync.dma_start(
                out=xs_sb[:], in_=xs.rearrange("(kt p) s -> p kt s", p=128)
            )
            nc.sync.dma_start(out=id_sb[:], in_=ident)
            nc.sync.dma_start(out=m0_sb[:], in_=m0)
            nc.sync.dma_start(out=m1_sb[:], in_=m1)
            nc.sync.dma_start(out=m2_sb[:], in_=m2)
            nc.sync.dma_start(out=ms_sb[:], in_=ms)
            wo_r = wo.rearrange("(h kt p) o -> p h kt o", p=128, h=2)
            nc.sync.dma_start(out=wo_sb[:, 0:4, :], in_=wo_r[:, 0])
            nc.sync.dma_start(out=wo_sb[:, 4:8, :], in_=wo_r[:, 1])

            q_sb = work.tile([64, H, SQ], MM_DT)  # q_T per head (d on partitions)
            k_sb = work.tile([64, 4, SKV], MM_DT)  # k_T per kv head
            ks_sb = work.tile([64, 4, NS], MM_DT)
            v_sb = work.tile([128, 3, 260], MM_DT)  # v rows, 65-stride heads
            vs_sb = work.tile([NS, 260], MM_DT)
            attn_sb = work.tile([128, 8, SQ], MM_DT)  # normalized attn_T

            # rotate PSUM->SBUF copies across Act/DVE (GPSIMD cannot read PSUM)
            _eng = [0]

            def copy_any(out, in_):
                e = _eng[0] % 2
                _eng[0] += 1
                if e == 0:
                    nc.scalar.activation(out, in_, COPY)
                else:
                    nc.vector.tensor_copy(out, in_)

            # ---- phase A: projections ----
            # ordered so xs-dependent work (ks, vs) comes last: xs arrives
            # later than xkv/wk in the DMA schedule.
            with tc.tile_pool(name="ps_proj", bufs=3, space="PSUM") as psp:
                for ot in range(2):  # k_T (256 kv channels)
                    kp = psp.tile([128, SKV], F32, tag="proj")
                    for kt in range(KT):
                        nc.tensor.matmul(
                            kp[:],
                            wk_sb[:, kt, bass.ts(ot, 128)],
                            xkv_sb[:, kt, :],
                            start=kt == 0,
                            stop=kt == KT - 1,
                        )
                    copy_any(k_sb[:, 2 * ot, :], kp[0:64, :])
                    copy_any(k_sb[:, 2 * ot + 1, :], kp[64:128, :])
                for mt in range(3):  # v rows
                    vp = psp.tile([128, 260], F32, tag="proj")
                    for kt in range(KT):
                        nc.tensor.matmul(
                            vp[:],
                            xkv_sb[:, kt, bass.ts(mt, 128)],
                            wv_sb[:, kt, :],
                            start=kt == 0,
                            stop=kt == KT - 1,
                        )
                    copy_any(v_sb[:, mt, :], vp[:])
                    ones_cols = v_sb[:, mt, :].rearrange("p (g c) -> p g c", g=4)[
                        :, :, 64
                    ]
                    nc.gpsimd.memset(ones_cols, 1.0)
                for ot in range(8):  # q_T
                    qp = psp.tile([128, SQ], F32, tag="proj")
                    for kt in range(KT):
                        nc.tensor.matmul(
                            qp[:],
                            wq_sb[:, kt, bass.ts(ot, 128)],
                            xkv_sb[:, kt, 128:384],
                            start=kt == 0,
                            stop=kt == KT - 1,
                        )
                    copy_any(q_sb[:, 2 * ot, :], qp[0:64, :])
                    copy_any(q_sb[:, 2 * ot + 1, :], qp[64:128, :])
                for ot in range(2):  # ks_T (strided keys; needs xs)
                    ksp = psp.tile([128, NS], F32, tag="proj")
                    for kt in range(KT):
                        nc.tensor.matmul(
                            ksp[:],
                            wk_sb[:, kt, bass.ts(ot, 128)],
                            xs_sb[:, kt, :],
                            start=kt == 0,
                            stop=kt == KT - 1,
                        )
                    copy_any(ks_sb[:, 2 * ot, :], ksp[0:64, :])
                    copy_any(ks_sb[:, 2 * ot + 1, :], ksp[64:128, :])
                vsp = psp.tile([NS, 260], F32, tag="proj")
                for kt in range(KT):
                    nc.tensor.matmul(
                        vsp[:],
                        xs_sb[:, kt, :],
                        wv_sb[:, kt, :],
                        start=kt == 0,
                        stop=kt == KT - 1,
                    )
                copy_any(vs_sb[:], vsp[:])
                vs_ones = vs_sb[:].rearrange("p (g c) -> p g c", g=4)[:, :, 64]
                nc.gpsimd.memset(vs_ones, 1.0)

            # ---- phase B: sparse attention per kv-head g ----
            def qh(h, c0, c1):  # q_T slice of head h, query cols [c0:c1)
                return q_sb[:, h, c0:c1]

            with (
                tc.tile_pool(name="ps_s", bufs=PSS_BUFS, space="PSUM") as pss,
                tc.tile_pool(name="ps_sw", bufs=PSSW_BUFS, space="PSUM") as pssw,
                tc.tile_pool(name="ps_pv", bufs=PSPV_BUFS, space="PSUM") as pspv,
                tc.tile_pool(name="ptiles", bufs=3) as pt,
                tc.tile_pool(name="small", bufs=8) as sm,
            ):
                for g in range(4):
                    # local j-tiles 0/2: mask preload then per-head scores
                    p_loc = []
                    for jt in (0, 2):
                        w0, w1 = WIN[jt]
                        win = w1 - w0
                        sp = pss.tile([128, 4 * win], F32, tag="sA")
                        msk = m0_sb if jt == 0 else m2_sb
                        nc.tensor.matmul(
                            sp[:], id_sb[:], msk[:], start=True, stop=False,
                            skip_group_check=True,
                        )
                        for hh in range(4):
                            nc.tensor.matmul(
                                sp[:, hh * win : (hh + 1) * win],
                                k_sb[:, g, bass.ts(jt, 128)],
                                qh(4 * g + hh, w0, w1),
                                start=False,
                                stop=True,
                                skip_group_check=True,
                            )
                        p = pt.tile([128, 4 * win], MM_DT, tag=f"p{jt}")
                        nc.scalar.activation(p[:], sp[:], EXP, scale=0.125)
                        p_loc.append(p)
                    p0, p2 = p_loc
                    # local j-tile 1: two single-bank halves through the sA ring
                    # (heads 2u, 2u+1 at cols [0:192],[192:384] of each half)
                    p1h = []
                    for u in range(2):
                        s1 = pss.tile([128, 384], F32, tag="sA")
                        nc.tensor.matmul(
                            s1[:], id_sb[:], m1_sb[:, 0:384],
                            start=True, stop=False, skip_group_check=True,
                        )
                        for hh in (0, 1):
                            nc.tensor.matmul(
                                s1[:, hh * 192 : (hh + 1) * 192],
                                k_sb[:, g, 128:256],
                                qh(4 * g + 2 * u + hh, 0, 192),
                                start=False,
                                stop=True,
                                skip_group_check=True,
                            )
                        p1 = pt.tile([128, 384], MM_DT, tag=f"p1{u}")
                        nc.scalar.activation(p1[:], s1[:], EXP, scale=0.125)
                        p1h.append(p1)
                    # strided keys (2 banks, double-buffered across g)
                    ss = pssw.tile([NS, 1024], F32, tag="sB")
                    nc.tensor.matmul(
                        ss[:, 0:512], id_sb[0:NS, 0:NS], ms_sb[:, 0:512],
                        start=True, stop=False, skip_group_check=True,
                    )
                    nc.tensor.matmul(
                        ss[:, 512:1024], id_sb[0:NS, 0:NS], ms_sb[:, 512:1024],
                        start=True, stop=False, skip_group_check=True,
                    )
                    for hh in range(4):
                        nc.tensor.matmul(
                            ss[:, hh * 256 : (hh + 1) * 256],
                            ks_sb[:, g, :],
                            qh(4 * g + hh, 0, 256),
                            start=False,
                            stop=True,
                            skip_group_check=True,
                        )
                    pstr = pt.tile([NS, 1024], MM_DT, tag="pstr")
                    nc.scalar.activation(pstr[:], ss[:], EXP, scale=0.125)

                    # PV + denominator (ones column) per head
                    vg = 65 * g
                    for hh in range(4):
                        h = 4 * g + hh
                        pv = pspv.tile([65, SQ], F32, tag="pv")
                        nc.tensor.matmul(
                            pv[:],
                            vs_sb[:, vg : vg + 65],
                            pstr[:, hh * 256 : (hh + 1) * 256],
                            start=True,
                            stop=False,
                        )
                        nc.tensor.matmul(
                            pv[:, 0:192],
                            v_sb[:, 1, vg : vg + 65],
                            p1h[hh // 2][:, (hh % 2) * 192 : (hh % 2) * 192 + 192],
                            start=False,
                            stop=False,
                        )
                        nc.tensor.matmul(
                            pv[:, 0:64],
                            v_sb[:, 0, vg : vg + 65],
                            p0[:, hh * 64 : (hh + 1) * 64],
                            start=False,
                            stop=False,
                        )
                        nc.tensor.matmul(
                            pv[:, 128:256],
                            v_sb[:, 2, vg : vg + 65],
                            p2[:, hh * 128 : (hh + 1) * 128],
                            start=False,
                            stop=True,
                        )
                        rt = sm.tile([1, SQ], F32, tag="recip")
                        nc.vector.reciprocal(rt[:], pv[64:65, :])
                        rep = sm.tile([64, SQ], F32, tag="rep")
                        nc.gpsimd.partition_broadcast(rep[:], rt[:], channels=64)
                        nc.vector.tensor_mul(
                            attn_sb[64 * (h % 2) : 64 * (h % 2) + 64, h // 2, :],
                            pv[0:64, :],
                            rep[:],
                        )

            # ---- phase C: output projection ----
            with (
                tc.tile_pool(name="ps_y", bufs=2, space="PSUM") as psy,
                tc.tile_pool(name="yout", bufs=2) as yo,
            ):
                # y rows: stationary = attn k-tile (reused for 2 N-chunks),
                # moving = wo 512-wide chunks; output [queries, model-dim].
                chunks = [(0, 0, 512), (0, 512, 512), (1, 0, 512), (1, 512, 512)]
                for i, (st, c0, cw) in enumerate(chunks):
                    yp = psy.tile([128, cw], F32, tag="y")
                    for kt in range(KT):
                        nc.tensor.matmul(
                            yp[:],
                            attn_sb[:, kt, bass.ts(st, 128)],
                            wo_sb[:, kt, c0 : c0 + cw],
                            start=kt == 0,
                            stop=kt == KT - 1,
                        )
                    ys = yo.tile([128, cw], MM_DT, tag="ysb")
                    if i % 2 == 0:
                        nc.scalar.activation(ys[:], yp[:], COPY)
                    else:
                        nc.vector.tensor_copy(ys[:], yp[:])
                    nc.sync.dma_start(
                        out=y[bass.ts(st, 128), c0 : c0 + cw], in_=ys[:]
                    )
    nc.compile()
    return nc


def host_prep(x, Wq, Wk, Wv, Wo):
    """Build per-core input maps (pure data reordering, no FLOPs)."""
    x2 = np.asarray(x, np.float32).reshape(S, D)
    xT = np.ascontiguousarray(x2.T)  # [D, S]
    xpad = np.zeros((D, 128 + S), np.float32)
    xpad[:, 128:] = xT
    xs = np.ascontiguousarray(xT[:, SIDX])  # [D, 44]
    wq = np.ascontiguousarray(np.asarray(Wq, np.float32).T)
    wk = np.ascontiguousarray(np.asarray(Wk, np.float32).T)
    wvT = np.asarray(Wv, np.float32).T  # [D, 256]
    wv = np.zeros((D, 260), np.float32)
    for g in range(4):
        wv[:, 65 * g : 65 * g + 64] = wvT[:, 64 * g : 64 * g + 64]
    wo = np.ascontiguousarray(np.asarray(Wo, np.float32).T)

    in_maps = []
    for c in range(NCORES):
        qs = SQ * c
        xkv = np.ascontiguousarray(xpad[:, qs : qs + SKV])
        ig = qs + np.arange(SQ)  # global query index per local col
        jg = qs - 128 + np.arange(SKV)  # global key index per local j row
        diff = ig[None, :] - jg[:, None]  # [384, 256]
        loc_valid = (diff >= 0) & (diff <= W - 1) & (jg[:, None] >= 0)
        masks = []
        for jt in range(3):
            w0, w1 = WIN[jt]
            base = loc_valid[128 * jt : 128 * (jt + 1), w0:w1]
            add = np.where(base, 0.0, MASKV).astype(np.float32)
            masks.append(np.ascontiguousarray(np.tile(add, (1, 4))))
        str_base = SIDX[:, None] <= ig[None, :] - W  # [44,256]
        add_s = np.where(str_base, 0.0, MASKV).astype(np.float32)
        msk_str = np.ascontiguousarray(np.tile(add_s, (1, 4)))
        in_maps.append(
            {
                "xkv": xkv.astype(NP_DT),
                "xs": xs.astype(NP_DT),
                "wq": wq.astype(NP_DT),
                "wk": wk.astype(NP_DT),
                "wv": wv.astype(NP_DT),
                "wo": wo.astype(NP_DT),
                "m0": masks[0].astype(NP_DT),
                "m1": masks[1].astype(NP_DT),
                "m2": masks[2].astype(NP_DT),
                "ms": msk_str.astype(NP_DT),
                "ident": np.eye(128, dtype=NP_DT),
            }
        )
    return in_maps


_NC_CACHE = {}


def get_nc():
    if "nc" not in _NC_CACHE:
        _NC_CACHE["nc"] = build_nc()
    return _NC_CACHE["nc"]


def kernel(x, Wq, Wk, Wv, Wo):
    nc = get_nc()
    in_maps = host_prep(x, Wq, Wk, Wv, Wo)
    res = run_bass_kernel_spmd(nc, in_maps, core_ids=list(range(NCORES)))
    yrows = np.concatenate([r["y"] for r in res.results], axis=0)  # [S, D]
    return np.ascontiguousarray(yrows).reshape(B, S, D).astype(np.float32)

